# revision 1
# baseline (speedup 1.0000x reference)
"""Trainium2 Bass kernel for an LSTM decoder with additive attention + large
vocab projection (nn_DecoderWithAttention).

Strategy: 8-way data parallel over batch (8 samples per core), zero
collectives. Recurrent state h is kept feature-major [D, B] and scaled by 2
(h' = 2h) so every sigmoid can be computed as sigma(x) = (1 + tanh(x/2)) / 2
on the ACT engine -- keeping all scalar-engine ops inside the single
"exp_and_others" activation table set (exp, tanh, identity, copy), avoiding
~2.7us table swaps per step. The compensating 0.5 factors are folded into
W_d/W_beta/W_hh/W_fc/W_ih2 on the host (exact: power-of-two scale).

Per-core per-step dataflow (s = 0..28):
  dec_projT [A,B]  = (0.5 Wd)^T @ h'          (PE, Wd stationary fp32)
  eT[A,(B,P)]      = tanh(enc_projT + dec_projT bcast)   (DVE add, ACT tanh)
  scores[1,(B,P)]  = wf^T @ eT                (PE, f32r streaming)
  alpha            = exp(scores)/sum          (ACT exp + fused accum)
  ctx[B,ENC]       = alpha @ feats            (PE, col-packed 4 samples/tile)
  ctx2             = (1 + tanh(zb/2)) * ctx   (= 2 sigmoid(zb) ctx)
  gates[B,4D]      = emb_pre[s] + ctx2@(W_ih2/2) + h'@(W_hh/2)
  LSTM cell in tanh form; h' = (1+tanh(o/2)) * tanh(c2)
fc phase: logits = H_all @ (W_fc/2) batched over all 29 steps, W_fc streamed
from HBM once. Output row t=0 stays zero (buffer pre-zeroed by the runner).
"""

import os
import sys

for _p in ("/opt/trn_rl_repo", os.path.expanduser("~/.axon_site/_ro/trn_rl_repo")):
    if os.path.isdir(_p) and _p not in sys.path:
        sys.path.insert(0, _p)

import numpy as np

import concourse.bass as bass
import concourse.tile as tile
from concourse import bacc, mybir
from concourse.bass_utils import run_bass_kernel_spmd
from concourse.masks import make_identity

F32 = mybir.dt.float32
F32R = mybir.dt.float32r

B, P, T = 64, 196, 30
E, D, A, ENC, V = 512, 512, 512, 512, 30000
NCORES = 8
BL = B // NCORES          # 8 samples per core
S = T - 1                 # 29 recurrent steps
BP = BL * P               # 1568
PH = P // 2               # 98
KC = 4                    # 128-row chunks per 512 feature dim
VCHUNK = 512
V_CHUNKS = [(i * VCHUNK, min(VCHUNK, V - i * VCHUNK))
            for i in range((V + VCHUNK - 1) // VCHUNK)]
FC_MT = [(1, 16), (17, 13)]   # H_all slots (t0, n_t); rows = n_t * BL

Tanh = mybir.ActivationFunctionType.Tanh
Exp = mybir.ActivationFunctionType.Exp
Ident = mybir.ActivationFunctionType.Identity
Add = mybir.AluOpType.add
Mult = mybir.AluOpType.mult


def r(ap):
    return ap.bitcast(F32R)


def build_program(with_biases=False):
    nc = bacc.Bacc(
        "TRN2",
        target_bir_lowering=False,
        debug=False,
        enable_asserts=False,
        num_devices=NCORES,
    )

    def din(name, shape, dt=F32):
        return nc.dram_tensor(name, list(shape), dt, kind="ExternalInput").ap()

    featsT_d = din("featsT", (128, KC, BP), F32R)          # [p,c,b*196+q] = feats[b,q,128c+p]
    feats_p256_d = din("feats_p256", (128, 2 * BL, ENC), F32R)  # (b,p) rows, P padded to 256
    pooledT_d = din("pooledT", (128, KC, BL), F32R)
    embT_d = din("embT", (128, KC, 32, BL), F32R)          # [p,c,t,b], t<29 used
    Wd_d = din("Wd", (128, KC, A))                   # 0.5*Wd_att rows
    wf_d = din("wf", (128, KC, 1), F32R)
    Wcat_d = din("Wcat", (128, 2 * KC, 4 * D), F32R)       # [0.5*W_ih[512:]; 0.5*W_hh] rows
    WihE_d = din("WihE", (128, KC, 4 * D), F32R)           # W_ih[:512] rows
    Wbeta_d = din("Wbeta", (128, KC, ENC), F32R)           # 0.5*W_beta rows
    Winih_d = din("Winih", (128, KC, D), F32R)
    Winic_d = din("Winic", (128, KC, D), F32R)
    We_d = din("We", (128, KC, A), F32R)             # We_att rows
    Wfc_d = din("Wfc", (128, KC, V), F32R)                 # 0.5*W_fc rows
    bihh_d = din("bihh", (1, 4 * D), F32R)                 # b_ih + b_hh
    binih_d = din("binih", (1, D), F32R)
    binic_d = din("binic", (1, D), F32R)
    bd_d = din("bd_til", (128, KC))                  # bd_att as [p, c]
    be_d = din("be_til", (128, KC))
    ones_d = din("ones128", (1, 128), F32R)
    if with_biases:
        bbeta_d = din("bbeta", (1, ENC), F32R)

    out_d = nc.dram_tensor("out_logits", [BL, T, V], F32, kind="ExternalOutput").ap()
    outT = out_d.rearrange("b t v -> t b v")

    with tile.TileContext(nc) as tc:
        with tc.tile_pool(name="const", bufs=1) as const:
            wd_sb = const.tile([128, KC, A], F32)
            wf_sb = const.tile([128, KC, 1], F32R)
            wcat_sb = const.tile([128, 2 * KC, 4 * D], F32R)
            wbeta_sb = const.tile([128, KC, ENC], F32R)
            enc_projT = const.tile([128, KC, BP], F32)
            emb_pre = const.tile([128, 2, 4 * D], F32R)
            H_all = const.tile([128, KC, T, BL], F32R)   # slot t: h' after t steps
            bd_sb = const.tile([128, KC], F32)
            be_sb = const.tile([128, KC], F32)
            ident = const.tile([BL, BL], F32)
            ident128 = const.tile([128, 128], F32)
            idsel = const.tile([128, 128], F32R)
            c_state = [const.tile([BL, D], F32, tag=f"cstate{i}", name=f"c_state{i}")
                       for i in range(2)]
            if with_biases:
                ones_lp = const.tile([1, 128], F32R)
                bbeta_sb = const.tile([1, ENC], F32R)
                nc.sync.dma_start(ones_lp[:], ones_d)
                nc.sync.dma_start(bbeta_sb[:], bbeta_d)

            nc.sync.dma_start(wd_sb[:], Wd_d)
            nc.sync.dma_start(wf_sb[:], wf_d)
            nc.sync.dma_start(wcat_sb[:], Wcat_d)
            nc.sync.dma_start(wbeta_sb[:], Wbeta_d)
            nc.sync.dma_start(bd_sb[:], bd_d)
            nc.sync.dma_start(be_sb[:], be_d)
            make_identity(nc, ident[:])
            make_identity(nc, ident128[:])
            nc.vector.tensor_copy(idsel[:], ident128[:])

            # ---------------- setup phase ----------------
            with tc.tile_pool(name="setup", bufs=1) as setup, \
                 tc.tile_pool(name="setup2", bufs=2) as setup2, \
                 tc.tile_pool(name="setup_ps", bufs=2, space="PSUM") as setup_ps:

                pooledT_sb = setup.tile([128, KC, BL], F32R)
                ones_sb = setup.tile([1, 128], F32R)
                bihh_sb = setup.tile([1, 4 * D], F32R)
                binih_sb = setup.tile([1, D], F32R)
                binic_sb = setup.tile([1, D], F32R)
                winih_sb = setup.tile([128, KC, D], F32R)
                winic_sb = setup.tile([128, KC, D], F32R)
                nc.sync.dma_start(pooledT_sb[:], pooledT_d)
                nc.sync.dma_start(ones_sb[:], ones_d)
                nc.sync.dma_start(bihh_sb[:], bihh_d)
                nc.sync.dma_start(binih_sb[:], binih_d)
                nc.sync.dma_start(binic_sb[:], binic_d)
                nc.sync.dma_start(winih_sb[:], Winih_d)
                nc.sync.dma_start(winic_sb[:], Winic_d)

                # h0/c0 (B-major): lhsT = pooledT chunks, rhs = W_init rows
                for which in range(2):
                    w_sb = winih_sb if which == 0 else winic_sb
                    b_row = binih_sb if which == 0 else binic_sb
                    ps = setup_ps.tile([BL, D], F32, tag="init_ps")
                    for kc in range(KC):
                        nc.tensor.matmul(ps[:], pooledT_sb[:, kc, :],
                                         w_sb[:, kc, :], start=(kc == 0), stop=False)
                    nc.tensor.matmul(ps[:], ones_sb[0:1, 0:BL], b_row[0:1, :],
                                     start=False, stop=True)
                    if which == 0:
                        h0 = setup.tile([BL, D], F32)
                        nc.scalar.activation(h0[:], ps[:], Tanh)
                        h0x2 = setup.tile([BL, D], F32)
                        nc.vector.tensor_scalar_mul(h0x2[:], h0[:], 2.0)
                        trps = setup_ps.tile([128, KC, BL], F32, tag="tr_ps")
                        for c in range(KC):
                            nc.tensor.transpose(trps[:, c, :],
                                                h0x2[:, c * 128:(c + 1) * 128],
                                                ident[:])
                        nc.vector.tensor_copy(H_all[:, :, 0, :], trps[:])
                    else:
                        nc.scalar.activation(c_state[0][:], ps[:], Tanh)

                # enc_projT = We^T @ featsT + be  (A-major)
                we_sb = setup.tile([128, KC, A], F32R)
                nc.sync.dma_start(we_sb[:], We_d)
                for n in range(KC):
                    nsl = bass.ts(n, BP // KC)  # 392 cols
                    ft_stage = setup2.tile([128, KC, BP // KC], F32R, tag="ftst")
                    nc.sync.dma_start(ft_stage[:], featsT_d[:, :, nsl])
                    for c in range(KC):
                        ps = setup_ps.tile([128, BP // KC], F32, tag="enc_ps")
                        for kc in range(KC):
                            nc.tensor.matmul(ps[:], we_sb[:, kc, bass.ts(c, 128)],
                                             ft_stage[:, kc, :],
                                             start=(kc == 0), stop=(kc == KC - 1))
                        nc.scalar.activation(enc_projT[:, c, nsl], ps[:], Ident,
                                             bias=be_sb[:, c:c + 1])

                # emb_pre = embT^T @ W_ih[:512] + (b_ih + b_hh), rows (t, b)
                # zero first: tile-1 rows 104..127 are never written but are
                # contracted against identity zeros in the selector matmul
                nc.vector.memset(emb_pre[:].bitcast(F32), 0.0)
                embT_sb = setup.tile([128, KC, 32, BL], F32R)
                nc.sync.dma_start(embT_sb[:], embT_d)
                for n in range(4):
                    nsl = bass.ts(n, 512)
                    wst = setup2.tile([128, KC, 512], F32R, tag="wihE")
                    nc.sync.dma_start(wst[:], WihE_d[:, :, nsl])
                    for mt, (t0, nt) in enumerate([(0, 16), (16, 13)]):
                        rows = nt * BL
                        ps = setup_ps.tile([128, 512], F32, tag="emb_ps")
                        for kc in range(KC):
                            nc.tensor.matmul(ps[0:rows, :],
                                             embT_sb[:, kc, t0:t0 + nt, :],
                                             wst[:, kc, :], start=(kc == 0),
                                             stop=False)
                        nc.tensor.matmul(ps[0:rows, :], ones_sb[0:1, 0:rows],
                                         bihh_sb[0:1, nsl], start=False, stop=True)
                        nc.vector.tensor_copy(emb_pre[0:rows, mt, nsl], ps[0:rows, :])

            # ---------------- recurrent loop ----------------
            with tc.tile_pool(name="lper", bufs=1) as lper, \
                 tc.tile_pool(name="big", bufs=2) as bigp, \
                 tc.tile_pool(name="t2k", bufs=6) as t2k, \
                 tc.tile_pool(name="tiny", bufs=3) as tinyp, \
                 tc.tile_pool(name="sm", bufs=2) as smp, \
                 tc.tile_pool(name="ps_small", bufs=1, space="PSUM") as ps_small, \
                 tc.tile_pool(name="ps_sc", bufs=1, space="PSUM") as ps_sc_pool, \
                 tc.tile_pool(name="ps_ctx", bufs=1, space="PSUM") as ps_ctx_pool, \
                 tc.tile_pool(name="ps_g", bufs=1, space="PSUM") as ps_g_pool:

                feats_p256 = lper.tile([128, 2 * BL, ENC], F32R)
                nc.sync.dma_start(feats_p256[:], feats_p256_d)
                alphaD = lper.tile([128, 2 * BL, BL], F32R)
                nc.vector.memset(alphaD[:].bitcast(F32), 0.0)

                for s in range(S):
                    hT = H_all[:, :, s, :]
                    c_prev = c_state[s % 2]
                    c_next = c_state[(s + 1) % 2]

                    # 1) dec_projT [128, KC, BL]  (fp32 stationary Wd)
                    ps_dec = ps_small.tile([128, KC, BL], F32, tag="small",
                                           name=f"psdec{s}")
                    for m in range(KC):
                        for kc in range(KC):
                            nc.tensor.matmul(ps_dec[:, m, :],
                                             wd_sb[:, kc, bass.ts(m, 128)],
                                             hT[:, kc, :].bitcast(F32),
                                             start=(kc == 0),
                                             stop=(kc == KC - 1))
                    decT = tinyp.tile([128, KC, BL], F32, tag="tiny",
                                      name=f"decT{s}")
                    for c in range(KC):
                        nc.scalar.activation(decT[:, c, :], ps_dec[:, c, :], Ident,
                                             bias=bd_sb[:, c:c + 1])

                    # 2..5) e = tanh(enc_proj + dec_proj); scores = wf^T e
                    # each 392-col chunk padded to a 512-element PSUM bank
                    ps_sc = ps_sc_pool.tile([1, KC, 512], F32, tag="sc",
                                            name=f"pssc{s}")
                    for c in range(KC):
                        sT = bigp.tile([128, BL, P], F32, tag="big", name=f"sT{s}_{c}")
                        nc.vector.tensor_tensor(
                            sT[:],
                            enc_projT[:, c, :].rearrange("p (b q) -> p b q", b=BL),
                            decT[:, c, :, None].broadcast_to([128, BL, P]), Add)
                        eT = bigp.tile([128, BP], F32R, tag="big", name=f"eT{s}_{c}")
                        nc.scalar.activation(eT[:], sT[:].rearrange("p b q -> p (b q)"),
                                             Tanh)
                        for n in range(KC):
                            nc.tensor.matmul(ps_sc[:, n, 0:BP // KC],
                                             wf_sb[:, c, :],
                                             eT[:, bass.ts(n, BP // KC)],
                                             start=(c == 0), stop=(c == KC - 1))

                    # 6) PSUM -> SBUF row, DMA-reshape to [BL, P]
                    sc_row = bigp.tile([1, KC, BP // KC], F32, tag="big",
                                       name=f"scrow{s}")
                    nc.vector.tensor_copy(sc_row[:], ps_sc[:, :, 0:BP // KC])
                    scores_sb = t2k.tile([BL, 256], F32, tag="t2k",
                                         name=f"scores{s}")
                    nc.vector.memset(scores_sb[:, P:256], 0.0)
                    # DMA copies the element stream linearly: [1,1568] -> [8,196]
                    nc.sync.dma_start(scores_sb[:, 0:P],
                                      sc_row[:].rearrange("o n q -> o (n q)"))

                    # 7..9) softmax, in place (|scores| < ~2, no max-shift)
                    sumexp = smp.tile([BL, 1], F32, tag="sm", name=f"sumexp{s}")
                    nc.scalar.activation(scores_sb[:, 0:P], scores_sb[:, 0:P], Exp,
                                         accum_out=sumexp[:])
                    rec = smp.tile([BL, 1], F32, tag="sm", name=f"rec{s}")
                    nc.vector.reciprocal(rec[:], sumexp[:])
                    nc.vector.tensor_scalar_mul(scores_sb[:, 0:P],
                                                scores_sb[:, 0:P], rec[:])
                    alpha = scores_sb

                    # 10) transpose alpha (P padded to 256) and scatter the
                    # columns into block-diagonal alphaD [128, 2*BL, BL]
                    ps_tr_a = ps_small.tile([128, 2, BL], F32, tag="small",
                                            name=f"pstra{s}")
                    for j in range(2):
                        nc.tensor.transpose(ps_tr_a[:, j, :],
                                            alpha[:, 128 * j:128 * (j + 1)],
                                            ident[:])
                    for b in range(BL):
                        nc.vector.tensor_copy(alphaD[:, 2 * b:2 * b + 2, b],
                                              ps_tr_a[:, :, b])

                    # 11) ctx[b,:] = sum_p alpha[b,p] feats[b,p,:] as one
                    # 16-K-tile accumulation -> contiguous [8, 512] PSUM rows
                    ps_ctx = ps_ctx_pool.tile([BL, ENC], F32, tag="ctx",
                                              name=f"psctx{s}")
                    for k in range(2 * BL):
                        nc.tensor.matmul(ps_ctx[:], alphaD[:, k, :],
                                         feats_p256[:, k, :],
                                         start=(k == 0), stop=(k == 2 * BL - 1))

                    # 13) z_beta = h' @ (0.5 W_beta); tau_b = tanh(0.5 z)
                    ps_b = ps_small.tile([BL, ENC], F32, tag="small",
                                         name=f"psb{s}")
                    for kc in range(KC):
                        nc.tensor.matmul(ps_b[:], hT[:, kc, :],
                                         wbeta_sb[:, kc, :],
                                         start=(kc == 0),
                                         stop=(not with_biases and kc == KC - 1))
                    if with_biases:
                        nc.tensor.matmul(ps_b[:], ones_lp[0:1, 0:BL],
                                         bbeta_sb[0:1, :], start=False, stop=True)
                    # tau_b and ctx2 as two compact 4-sample tiles (SBUF APs must
                    # start at partition 0; PSUM reads at odd bases are fine)
                    ctx2c = []
                    for g in range(2):
                        taub = t2k.tile([BL, ENC], F32, tag="t2k", name=f"taub{s}")
                    nc.scalar.activation(taub[:], ps_b[:], Tanh, scale=0.5)
                    ctx2 = t2k.tile([BL, ENC], F32, tag="t2k", name=f"ctx2{s}")
                    nc.vector.scalar_tensor_tensor(ctx2[:], taub[:], 1.0,
                                                   ps_ctx[:], op0=Add, op1=Mult)

                    # 16) ctx2T [128, KC, BL]
                    ps_tr_c = ps_small.tile([128, KC, BL], F32, tag="small",
                                            name=f"pstrc{s}")
                    for c in range(KC):
                        nc.tensor.transpose(ps_tr_c[:, c, :], ctx2[:, bass.ts(c, 128)],
                                            ident[:])
                    ctx2T = tinyp.tile([128, KC, BL], F32R, tag="tiny",
                                       name=f"ctx2T{s}")
                    nc.vector.tensor_copy(ctx2T[:], ps_tr_c[:])

                    # 17) gates = ctx2 @ 0.5W_ih2 + h' @ 0.5W_hh + emb_pre
                    # gate preactivations; emb_pre added via identity-selector
                    # matmul so the activations read straight from PSUM
                    erow = (s % 16) * BL
                    gacts = []  # tau_i, tau_f, gt, tau_o
                    for n in range(4):
                        nsl = bass.ts(n, D)
                        ps_g = ps_g_pool.tile([BL, D], F32, tag="g",
                                              name=f"psg{s}_{n}")
                        for kc in range(KC):
                            nc.tensor.matmul(ps_g[:], ctx2T[:, kc, :],
                                             wcat_sb[:, kc, nsl],
                                             start=(kc == 0), stop=False)
                        for kc in range(KC):
                            nc.tensor.matmul(ps_g[:], hT[:, kc, :],
                                             wcat_sb[:, KC + kc, nsl],
                                             start=False, stop=False)
                        nc.tensor.matmul(ps_g[:],
                                         idsel[:, erow:erow + BL],
                                         emb_pre[:, s // 16, nsl],
                                         start=False, stop=True)
                        ga = t2k.tile([BL, D], F32, tag="t2k", name=f"ga{s}_{n}")
                        nc.scalar.activation(ga[:], ps_g[:], Tanh,
                                             scale=(1.0 if n == 2 else 0.5))
                        gacts.append(ga)
                    tau_i, tau_f, gt, tau_o = gacts

                    # 22..27) LSTM cell in tanh form
                    t_a = t2k.tile([BL, D], F32, tag="t2k", name=f"ta{s}")
                    nc.vector.scalar_tensor_tensor(t_a[:], tau_f[:], 1.0,
                                                   c_prev[:], op0=Add, op1=Mult)
                    t_b = t2k.tile([BL, D], F32, tag="t2k", name=f"tb{s}")
                    nc.vector.scalar_tensor_tensor(t_b[:], tau_i[:], 1.0,
                                                   gt[:], op0=Add, op1=Mult)
                    s2 = t2k.tile([BL, D], F32, tag="t2k", name=f"s2{s}")
                    nc.vector.tensor_add(s2[:], t_a[:], t_b[:])
                    nc.vector.tensor_scalar_mul(c_next[:], s2[:], 0.5)
                    tc2 = t2k.tile([BL, D], F32, tag="t2k", name=f"tc2{s}")
                    nc.scalar.activation(tc2[:], s2[:], Tanh, scale=0.5)
                    h2 = t2k.tile([BL, D], F32, tag="t2k", name=f"h2{s}")
                    nc.vector.scalar_tensor_tensor(h2[:], tau_o[:], 1.0, tc2[:],
                                                   op0=Add, op1=Mult)

                    # 28) h2 -> H_all[:, :, s+1, :]
                    ps_tr_h = ps_small.tile([128, KC, BL], F32, tag="small",
                                            name=f"pstrh{s}")
                    for c in range(KC):
                        nc.tensor.transpose(ps_tr_h[:, c, :], h2[:, bass.ts(c, 128)],
                                            ident[:])
                    nc.vector.tensor_copy(H_all[:, :, s + 1, :], ps_tr_h[:])

            # ---------------- fc phase ----------------
            with tc.tile_pool(name="fcw", bufs=3) as fcw, \
                 tc.tile_pool(name="fco", bufs=4) as fco, \
                 tc.tile_pool(name="ps_fc", bufs=4, space="PSUM") as ps_fc_pool:
                # output row t=0 is defined to be zeros
                zt = fco.tile([1, 400], F32, tag="zt")
                nc.vector.memset(zt[:], 0.0)
                zsrc = zt[0:1, 0:400]
                zsrc = bass.AP(tensor=zsrc.tensor, offset=zsrc.offset,
                               ap=[list(zsrc.ap[0]), [0, BL * V // 400], [1, 400]])
                nc.sync.dma_start(out_d[:, 0, :], zsrc)
                for ci, (v0, vn) in enumerate(V_CHUNKS):
                    wst = fcw.tile([128, KC, VCHUNK], F32R, tag="fcw",
                                   name=f"fcw{ci}")
                    nc.sync.dma_start(wst[:, :, 0:vn], Wfc_d[:, :, v0:v0 + vn])
                    for (t0, nt) in FC_MT:
                        rows = nt * BL
                        ps = ps_fc_pool.tile([128, VCHUNK], F32, tag="fc",
                                             name=f"psfc{ci}_{t0}")
                        for kc in range(KC):
                            nc.tensor.matmul(ps[0:rows, 0:vn],
                                             H_all[:, kc, t0:t0 + nt, :],
                                             wst[:, kc, 0:vn],
                                             start=(kc == 0), stop=(kc == KC - 1))
                        ost = fco.tile([128, VCHUNK], F32, tag="fco",
                                       name=f"fco{ci}_{t0}")
                        nc.vector.tensor_copy(ost[0:rows, 0:vn], ps[0:rows, 0:vn])
                        # linear stream: src rows (t,b) match dst dims [t][b][v]
                        nc.sync.dma_start(outT[t0:t0 + nt, :, v0:v0 + vn],
                                          ost[0:rows, 0:vn])

    nc.compile()
    return nc


def _prep_core_inputs(inputs, k):
    """Host-side marshalling for core k (samples 8k..8k+8)."""
    f32 = np.float32
    bs = slice(BL * k, BL * (k + 1))
    feats = np.ascontiguousarray(inputs["encoder_feats"][bs]).astype(f32)
    pooled = np.ascontiguousarray(inputs["encoder_pooled"][bs]).astype(f32)
    caps = np.asarray(inputs["captions"][bs])

    d = {}
    ft = feats.transpose(2, 0, 1).reshape(KC, 128, BP)
    d["featsT"] = np.ascontiguousarray(ft.transpose(1, 0, 2))
    fp = np.zeros((128, 2 * BL, ENC), f32)
    for b in range(BL):
        fp[0:128, 2 * b] = feats[b, 0:128]
        fp[0:P - 128, 2 * b + 1] = feats[b, 128:P]
    d["feats_p256"] = fp
    d["pooledT"] = np.ascontiguousarray(
        pooled.T.reshape(KC, 128, BL).transpose(1, 0, 2))
    emb = np.asarray(inputs["emb_table"], f32)[caps[:, :S]]      # (8, 29, 512)
    et = np.zeros((128, KC, 32, BL), f32)
    g = emb.transpose(2, 1, 0).reshape(KC, 128, S, BL)
    et[:, :, 0:S, :] = g.transpose(1, 0, 2, 3)
    d["embT"] = et
    return d


def _prep_shared_inputs(inputs):
    f32 = np.float32

    def rows(w):  # (512, N) -> [128, 4, N]
        return np.ascontiguousarray(
            np.asarray(w, f32).reshape(KC, 128, -1).transpose(1, 0, 2))

    d = {}
    d["Wd"] = rows(0.5 * np.asarray(inputs["Wd_att"], f32))
    d["wf"] = rows(inputs["wf_att"])
    wih2 = 0.5 * np.asarray(inputs["W_ih"][E:], f32)
    whh = 0.5 * np.asarray(inputs["W_hh"], f32)
    wcat = np.concatenate([wih2, whh], 0)                        # (1024, 2048)
    d["Wcat"] = np.ascontiguousarray(
        wcat.reshape(2 * KC, 128, 4 * D).transpose(1, 0, 2))
    d["WihE"] = rows(inputs["W_ih"][:E])
    d["Wbeta"] = rows(0.5 * np.asarray(inputs["W_beta"], f32))
    d["Winih"] = rows(inputs["W_init_h"])
    d["Winic"] = rows(inputs["W_init_c"])
    d["We"] = rows(inputs["We_att"])
    d["Wfc"] = rows(0.5 * np.asarray(inputs["W_fc"], f32))
    d["bihh"] = (np.asarray(inputs["b_ih"], f32)
                 + np.asarray(inputs["b_hh"], f32)).reshape(1, -1)
    d["binih"] = np.asarray(inputs["b_init_h"], f32).reshape(1, -1)
    d["binic"] = np.asarray(inputs["b_init_c"], f32).reshape(1, -1)
    d["bd_til"] = np.ascontiguousarray(
        np.asarray(inputs["bd_att"], f32).reshape(KC, 128).T)
    d["be_til"] = np.ascontiguousarray(
        np.asarray(inputs["be_att"], f32).reshape(KC, 128).T)
    d["ones128"] = np.ones((1, 128), f32)
    return d


_NC_CACHE = {}


def _get_program(with_biases=False):
    if with_biases not in _NC_CACHE:
        _NC_CACHE[with_biases] = build_program(with_biases)
    return _NC_CACHE[with_biases]


def run_on_device(inputs, trace=False, **kw):
    with_biases = bool(np.any(np.asarray(inputs["b_beta"], np.float32)))
    nc = _get_program(with_biases)
    shared = _prep_shared_inputs(inputs)
    if with_biases:
        shared["bbeta"] = np.asarray(inputs["b_beta"], np.float32).reshape(1, -1)
    in_maps = []
    for k in range(NCORES):
        m = dict(shared)
        m.update(_prep_core_inputs(inputs, k))
        in_maps.append(m)
    return run_bass_kernel_spmd(nc, in_maps, list(range(NCORES)), trace=trace, **kw)


def kernel(**inputs) -> np.ndarray:
    res = run_on_device(inputs)
    parts = [res.results[k]["out_logits"] for k in range(NCORES)]
    out = np.concatenate(parts, axis=0)
    b_fc = np.asarray(inputs["b_fc"], np.float32).reshape(1, 1, V)
    out[:, 1:, :] += b_fc
    return out



# revision 3
# speedup vs baseline: 47.1174x; 47.1174x over previous
"""Trainium2 Bass kernel for an LSTM decoder with additive attention + large
vocab projection (nn_DecoderWithAttention).

Strategy: 8-way data parallel over batch (8 samples per core), zero
collectives. Recurrent state h is kept feature-major [D, B] and scaled by 2
(h' = 2h) so every sigmoid can be computed as sigma(x) = (1 + tanh(x/2)) / 2
on the ACT engine -- keeping all scalar-engine ops inside the single
"exp_and_others" activation table set (exp, tanh, identity, copy), avoiding
~2.7us table swaps per step. The compensating 0.5 factors are folded into
W_d/W_beta/W_hh/W_fc/W_ih2 on the host (exact: power-of-two scale).

Per-core per-step dataflow (s = 0..28):
  dec_projT [A,B]  = (0.5 Wd)^T @ h'          (PE, bf16)
  eT[A,(B,P)]      = tanh(enc_projT + dec_projT bcast)   (DVE add, ACT tanh)
  scores[1,(B,P)]  = wf^T @ eT                (PE, f32r streaming)
  alpha            = exp(scores)/sum          (ACT exp + fused accum)
  ctx[B,ENC]       = alpha @ feats            (PE, col-packed 4 samples/tile)
  ctx2             = (1 + tanh(zb/2)) * ctx   (= 2 sigmoid(zb) ctx)
  gates[B,4D]      = emb_pre[s] + ctx2@(W_ih2/2) + h'@(W_hh/2)
  LSTM cell in tanh form; h' = (1+tanh(o/2)) * tanh(c2)
fc phase: logits = H_all @ (W_fc/2), batched over steps. W_fc is repacked on
the host into per-vocab-chunk contiguous bf16 blocks (60 x [128,4,500]) so
each chunk load is one contiguous ~0.5MB DMA. Chunks for rows 1..16 are
interleaved into loop steps 16..27 (keeps PE warm + hides the DMA); rows
17..29 run after the loop. Output goes to a chunk-major DRAM buffer
[60, 232, 500] (fully contiguous writes); the host reassembles [B, T, V].
"""

import os
import sys

for _p in ("/opt/trn_rl_repo", os.path.expanduser("~/.axon_site/_ro/trn_rl_repo")):
    if os.path.isdir(_p) and _p not in sys.path:
        sys.path.insert(0, _p)

import ml_dtypes
import numpy as np

import concourse.bass as bass
import concourse.tile as tile
from concourse import bacc, mybir
from concourse.bass_utils import run_bass_kernel_spmd
from concourse.masks import make_identity

F32 = mybir.dt.float32
F32R = mybir.dt.float32r
BF16 = mybir.dt.bfloat16

B, P, T = 64, 196, 30
E, D, A, ENC, V = 512, 512, 512, 512, 30000
NCORES = 8
BL = B // NCORES          # 8 samples per core
S = T - 1                 # 29 recurrent steps
BP = BL * P               # 1568
KC = 4                    # 128-row chunks per 512 feature dim
VCHUNK = 500              # 30000 = 60 * 500, fits one PSUM bank
NVC = V // VCHUNK         # 60
FC_G0 = (1, 16)           # H rows 1..16 ready after step 15
FC_G1 = (17, 13)          # rows 17..29 ready after the loop
FC_ROWS = S * BL          # 232 output rows in the chunk buffer

Tanh = mybir.ActivationFunctionType.Tanh
Exp = mybir.ActivationFunctionType.Exp
Ident = mybir.ActivationFunctionType.Identity
Add = mybir.AluOpType.add
Mult = mybir.AluOpType.mult


def r(ap):
    return ap.bitcast(F32R)


def build_program(with_biases=False):
    nc = bacc.Bacc(
        "TRN2",
        target_bir_lowering=False,
        debug=False,
        enable_asserts=False,
        num_devices=NCORES,
    )

    def din(name, shape, dt=F32):
        return nc.dram_tensor(name, list(shape), dt, kind="ExternalInput").ap()

    featsT_d = din("featsT", (128, KC, BP), F32R)          # [p,c,b*196+q] = feats[b,q,128c+p]
    feats_p256_d = din("feats_p256", (128, 2 * BL, ENC), F32R)  # (b,p) rows, P padded to 256
    pooledT_d = din("pooledT", (128, KC, BL), F32R)
    embT_d = din("embT", (128, KC, 32, BL), F32R)          # [p,c,t,b], t<29 used
    Wd_d = din("Wd", (128, KC, A), BF16)                   # 0.5*Wd_att rows
    wf_d = din("wf", (128, KC, 1), F32R)
    Wcat_d = din("Wcat", (128, 2 * KC, 4 * D), BF16)       # [0.5*W_ih[512:]; 0.5*W_hh] rows
    WihE_d = din("WihE", (128, KC, 4 * D), F32R)           # W_ih[:512] rows
    Wbeta_d = din("Wbeta", (128, KC, ENC), BF16)           # 0.5*W_beta rows
    Winih_d = din("Winih", (128, KC, D), F32R)
    Winic_d = din("Winic", (128, KC, D), F32R)
    We_d = din("We", (128, KC, A), F32R)             # We_att rows
    Wfc_d = din("Wfc", (NVC, 128, KC, VCHUNK), BF16)       # 0.5*W_fc, chunk-contig
    bihh_d = din("bihh", (1, 4 * D), F32R)                 # b_ih + b_hh
    binih_d = din("binih", (1, D), F32R)
    binic_d = din("binic", (1, D), F32R)
    bd_d = din("bd_til", (128, KC))                  # bd_att as [p, c]
    be_d = din("be_til", (128, KC))
    ones_d = din("ones128", (1, 128), F32R)
    if with_biases:
        bbeta_d = din("bbeta", (1, ENC), F32R)

    # chunk-major output buffer: [chunk, (t,b) row, v] -- contiguous writes
    out_d = nc.dram_tensor("out_chunks", [NVC, FC_ROWS, VCHUNK], F32,
                           kind="ExternalOutput").ap()

    with tile.TileContext(nc) as tc:
        with tc.tile_pool(name="const", bufs=1) as const:
            wd_sb = const.tile([128, KC, A], BF16)
            wf_sb = const.tile([128, KC, 1], F32R)
            wcat_sb = const.tile([128, 2 * KC, 4 * D], BF16)
            wbeta_sb = const.tile([128, KC, ENC], BF16)
            enc_projT = const.tile([128, KC, BP], F32)
            emb_pre = const.tile([128, 2, 4 * D], BF16)
            H_all = const.tile([128, KC, T, BL], BF16)   # slot t: h' after t steps
            bd_sb = const.tile([128, KC], F32)
            be_sb = const.tile([128, KC], F32)
            ident = const.tile([BL, BL], F32)
            ident128 = const.tile([128, 128], F32)
            idsel = const.tile([128, 128], BF16)
            c_state = [const.tile([BL, D], F32, tag=f"cstate{i}", name=f"c_state{i}")
                       for i in range(2)]
            if with_biases:
                ones_lp = const.tile([1, 128], F32R)
                bbeta_sb = const.tile([1, ENC], F32R)
                nc.sync.dma_start(ones_lp[:], ones_d)
                nc.sync.dma_start(bbeta_sb[:], bbeta_d)

            nc.sync.dma_start(wd_sb[:], Wd_d)
            nc.sync.dma_start(wf_sb[:], wf_d)
            nc.sync.dma_start(wcat_sb[:], Wcat_d)
            nc.sync.dma_start(wbeta_sb[:], Wbeta_d)
            nc.sync.dma_start(bd_sb[:], bd_d)
            nc.sync.dma_start(be_sb[:], be_d)
            make_identity(nc, ident[:])
            make_identity(nc, ident128[:])
            nc.vector.tensor_copy(idsel[:], ident128[:])

            # ---------------- setup phase ----------------
            with tc.tile_pool(name="setup", bufs=1) as setup, \
                 tc.tile_pool(name="setup_ps", bufs=2, space="PSUM") as setup_ps:

                pooledT_sb = setup.tile([128, KC, BL], F32R)
                ones_sb = setup.tile([1, 128], F32R)
                bihh_sb = setup.tile([1, 4 * D], F32R)
                binih_sb = setup.tile([1, D], F32R)
                binic_sb = setup.tile([1, D], F32R)
                winih_sb = setup.tile([128, KC, D], F32R)
                winic_sb = setup.tile([128, KC, D], F32R)
                featsT_sb = setup.tile([128, KC, BP], F32R)
                embT_sb = setup.tile([128, KC, 32, BL], F32R)
                wihE_sb = setup.tile([128, KC, 4 * D], F32R)
                we_sb = setup.tile([128, KC, A], F32R)
                nc.sync.dma_start(pooledT_sb[:], pooledT_d)
                nc.sync.dma_start(ones_sb[:], ones_d)
                nc.sync.dma_start(bihh_sb[:], bihh_d)
                nc.sync.dma_start(binih_sb[:], binih_d)
                nc.sync.dma_start(binic_sb[:], binic_d)
                nc.sync.dma_start(winih_sb[:], Winih_d)
                nc.sync.dma_start(winic_sb[:], Winic_d)
                nc.sync.dma_start(featsT_sb[:], featsT_d)
                nc.sync.dma_start(embT_sb[:], embT_d)
                nc.sync.dma_start(wihE_sb[:], WihE_d)
                nc.sync.dma_start(we_sb[:], We_d)

                # h0/c0 (B-major): lhsT = pooledT chunks, rhs = W_init rows
                for which in range(2):
                    w_sb = winih_sb if which == 0 else winic_sb
                    b_row = binih_sb if which == 0 else binic_sb
                    ps = setup_ps.tile([BL, D], F32, tag="init_ps")
                    for kc in range(KC):
                        nc.tensor.matmul(ps[:], pooledT_sb[:, kc, :],
                                         w_sb[:, kc, :], start=(kc == 0), stop=False)
                    nc.tensor.matmul(ps[:], ones_sb[0:1, 0:BL], b_row[0:1, :],
                                     start=False, stop=True)
                    if which == 0:
                        h0 = setup.tile([BL, D], F32)
                        nc.scalar.activation(h0[:], ps[:], Tanh)
                        h0x2 = setup.tile([BL, D], F32)
                        nc.vector.tensor_scalar_mul(h0x2[:], h0[:], 2.0)
                        trps = setup_ps.tile([128, KC, BL], F32, tag="tr_ps")
                        for c in range(KC):
                            nc.tensor.transpose(trps[:, c, :],
                                                h0x2[:, c * 128:(c + 1) * 128],
                                                ident[:])
                        nc.vector.tensor_copy(H_all[:, :, 0, :], trps[:])
                    else:
                        nc.scalar.activation(c_state[0][:], ps[:], Tanh)

                # enc_projT = We^T @ featsT + be  (A-major)
                for n in range(KC):
                    nsl = bass.ts(n, BP // KC)  # 392 cols
                    for c in range(KC):
                        ps = setup_ps.tile([128, BP // KC], F32, tag="enc_ps")
                        for kc in range(KC):
                            nc.tensor.matmul(ps[:], we_sb[:, kc, bass.ts(c, 128)],
                                             featsT_sb[:, kc, nsl],
                                             start=(kc == 0), stop=(kc == KC - 1))
                        nc.scalar.activation(enc_projT[:, c, nsl], ps[:], Ident,
                                             bias=be_sb[:, c:c + 1])

                # emb_pre = embT^T @ W_ih[:512] + (b_ih + b_hh), rows (t, b)
                # zero first: tile-1 rows 104..127 are never written but are
                # contracted against identity zeros in the selector matmul
                nc.vector.memset(emb_pre[:], 0.0)
                for n in range(4):
                    nsl = bass.ts(n, 512)
                    for mt, (t0, nt) in enumerate([(0, 16), (16, 13)]):
                        rows = nt * BL
                        ps = setup_ps.tile([128, 512], F32, tag="emb_ps")
                        for kc in range(KC):
                            nc.tensor.matmul(ps[0:rows, :],
                                             embT_sb[:, kc, t0:t0 + nt, :],
                                             wihE_sb[:, kc, nsl], start=(kc == 0),
                                             stop=False)
                        nc.tensor.matmul(ps[0:rows, :], ones_sb[0:1, 0:rows],
                                         bihh_sb[0:1, nsl], start=False, stop=True)
                        nc.vector.tensor_copy(emb_pre[0:rows, mt, nsl], ps[0:rows, :])

            # ---------------- recurrent loop + interleaved fc ----------------
            with tc.tile_pool(name="lper", bufs=1) as lper, \
                 tc.tile_pool(name="big", bufs=2) as bigp, \
                 tc.tile_pool(name="t2k", bufs=6) as t2k, \
                 tc.tile_pool(name="tiny", bufs=3) as tinyp, \
                 tc.tile_pool(name="sm", bufs=2) as smp, \
                 tc.tile_pool(name="fcw", bufs=6) as fcw, \
                 tc.tile_pool(name="fco", bufs=4) as fco, \
                 tc.tile_pool(name="ps_small", bufs=1, space="PSUM") as ps_small, \
                 tc.tile_pool(name="ps_sc", bufs=1, space="PSUM") as ps_sc_pool, \
                 tc.tile_pool(name="ps_ctx", bufs=1, space="PSUM") as ps_ctx_pool, \
                 tc.tile_pool(name="ps_g", bufs=1, space="PSUM") as ps_g_pool, \
                 tc.tile_pool(name="ps_fc", bufs=1, space="PSUM") as ps_fc_pool:

                feats_p256 = lper.tile([128, 2 * BL, ENC], F32R)
                nc.sync.dma_start(feats_p256[:], feats_p256_d)
                alphaD = lper.tile([128, 2 * BL, BL], F32R)
                nc.vector.memset(alphaD[:].bitcast(F32), 0.0)

                def emit_fc_chunk(ci, t0, nt, r0):
                    rows = nt * BL
                    wst = fcw.tile([128, KC, VCHUNK], BF16, tag="fcw",
                                   name=f"fcw{ci}_{t0}")
                    nc.sync.dma_start(wst[:], Wfc_d[ci])
                    ps = ps_fc_pool.tile([128, VCHUNK], F32, tag="fc",
                                         name=f"psfc{ci}_{t0}")
                    for kc in range(KC):
                        nc.tensor.matmul(ps[0:rows, :],
                                         H_all[:, kc, t0:t0 + nt, :],
                                         wst[:, kc, :],
                                         start=(kc == 0), stop=(kc == KC - 1))
                    ost = fco.tile([128, VCHUNK], F32, tag="fco",
                                   name=f"fco{ci}_{t0}")
                    nc.vector.tensor_copy(ost[0:rows, :], ps[0:rows, :])
                    nc.sync.dma_start(out_d[ci, r0:r0 + rows, :], ost[0:rows, :])

                fc_g0_next = 0
                for s in range(S):
                    hT = H_all[:, :, s, :]
                    c_prev = c_state[s % 2]
                    c_next = c_state[(s + 1) % 2]

                    # 1) dec_projT [128, KC, BL]  (bf16 stationary Wd)
                    ps_dec = ps_small.tile([128, KC, BL], F32, tag="small",
                                           name=f"psdec{s}")
                    for m in range(KC):
                        for kc in range(KC):
                            nc.tensor.matmul(ps_dec[:, m, :],
                                             wd_sb[:, kc, bass.ts(m, 128)],
                                             hT[:, kc, :],
                                             start=(kc == 0),
                                             stop=(kc == KC - 1))
                    decT = tinyp.tile([128, KC, BL], F32, tag="tiny",
                                      name=f"decT{s}")
                    for c in range(KC):
                        nc.scalar.activation(decT[:, c, :], ps_dec[:, c, :], Ident,
                                             bias=bd_sb[:, c:c + 1])

                    # 2..5) e = tanh(enc_proj + dec_proj); scores = wf^T e
                    ps_sc = ps_sc_pool.tile([1, KC, 512], F32, tag="sc",
                                            name=f"pssc{s}")
                    for c in range(KC):
                        sT = bigp.tile([128, BL, P], F32, tag="big", name=f"sT{s}_{c}")
                        nc.vector.tensor_tensor(
                            sT[:],
                            enc_projT[:, c, :].rearrange("p (b q) -> p b q", b=BL),
                            decT[:, c, :, None].broadcast_to([128, BL, P]), Add)
                        eT = bigp.tile([128, BP], F32R, tag="big", name=f"eT{s}_{c}")
                        nc.scalar.activation(eT[:], sT[:].rearrange("p b q -> p (b q)"),
                                             Tanh)
                        for n in range(KC):
                            nc.tensor.matmul(ps_sc[:, n, 0:BP // KC],
                                             wf_sb[:, c, :],
                                             eT[:, bass.ts(n, BP // KC)],
                                             start=(c == 0), stop=(c == KC - 1))

                    # 6) PSUM -> SBUF row, DMA-reshape to [BL, P]
                    sc_row = bigp.tile([1, KC, BP // KC], F32, tag="big",
                                       name=f"scrow{s}")
                    nc.vector.tensor_copy(sc_row[:], ps_sc[:, :, 0:BP // KC])
                    scores_sb = t2k.tile([BL, 256], F32, tag="t2k",
                                         name=f"scores{s}")
                    nc.vector.memset(scores_sb[:, P:256], 0.0)
                    # DMA copies the element stream linearly: [1,1568] -> [8,196]
                    nc.sync.dma_start(scores_sb[:, 0:P],
                                      sc_row[:].rearrange("o n q -> o (n q)"))

                    # 7..9) softmax, in place (|scores| < ~2, no max-shift)
                    sumexp = smp.tile([BL, 1], F32, tag="sm", name=f"sumexp{s}")
                    nc.scalar.activation(scores_sb[:, 0:P], scores_sb[:, 0:P], Exp,
                                         accum_out=sumexp[:])
                    rec = smp.tile([BL, 1], F32, tag="sm", name=f"rec{s}")
                    nc.vector.reciprocal(rec[:], sumexp[:])
                    nc.vector.tensor_scalar_mul(scores_sb[:, 0:P],
                                                scores_sb[:, 0:P], rec[:])
                    alpha = scores_sb

                    # 10) transpose alpha (P padded to 256) and scatter the
                    # columns into block-diagonal alphaD [128, 2*BL, BL]
                    ps_tr_a = ps_small.tile([128, 2, BL], F32, tag="small",
                                            name=f"pstra{s}")
                    for j in range(2):
                        nc.tensor.transpose(ps_tr_a[:, j, :],
                                            alpha[:, 128 * j:128 * (j + 1)],
                                            ident[:])
                    # single strided copy: dst[p, 2b+j, b] <- src[p, j, b]
                    aD = alphaD[:].rearrange("p k b -> p (k b)")
                    dst = bass.AP(tensor=aD.tensor, offset=aD.offset,
                                  ap=[list(aD.ap[0]), [BL, 2], [2 * BL + 1, BL]])
                    nc.vector.tensor_copy(dst, ps_tr_a[:])

                    # 11) ctx[b,:] = sum_p alpha[b,p] feats[b,p,:] as one
                    # 16-K-tile accumulation -> contiguous [8, 512] PSUM rows
                    ps_ctx = ps_ctx_pool.tile([BL, ENC], F32, tag="ctx",
                                              name=f"psctx{s}")
                    for k in range(2 * BL):
                        nc.tensor.matmul(ps_ctx[:], alphaD[:, k, :],
                                         feats_p256[:, k, :],
                                         start=(k == 0), stop=(k == 2 * BL - 1))

                    # 13) z_beta = h' @ (0.5 W_beta); tau_b = tanh(0.5 z)
                    ps_b = ps_small.tile([BL, ENC], F32, tag="small",
                                         name=f"psb{s}")
                    for kc in range(KC):
                        nc.tensor.matmul(ps_b[:], hT[:, kc, :],
                                         wbeta_sb[:, kc, :],
                                         start=(kc == 0),
                                         stop=(not with_biases and kc == KC - 1))
                    if with_biases:
                        nc.tensor.matmul(ps_b[:], ones_lp[0:1, 0:BL],
                                         bbeta_sb[0:1, :], start=False, stop=True)
                    taub = t2k.tile([BL, ENC], F32, tag="t2k", name=f"taub{s}")
                    nc.scalar.activation(taub[:], ps_b[:], Tanh, scale=0.5)
                    ctx2 = t2k.tile([BL, ENC], F32, tag="t2k", name=f"ctx2{s}")
                    nc.vector.scalar_tensor_tensor(ctx2[:], taub[:], 1.0,
                                                   ps_ctx[:], op0=Add, op1=Mult)

                    # 16) ctx2T [128, KC, BL]
                    ps_tr_c = ps_small.tile([128, KC, BL], F32, tag="small",
                                            name=f"pstrc{s}")
                    for c in range(KC):
                        nc.tensor.transpose(ps_tr_c[:, c, :], ctx2[:, bass.ts(c, 128)],
                                            ident[:])
                    ctx2T = tinyp.tile([128, KC, BL], BF16, tag="tiny",
                                       name=f"ctx2T{s}")
                    nc.vector.tensor_copy(ctx2T[:], ps_tr_c[:])

                    # 17) gates = ctx2 @ 0.5W_ih2 + h' @ 0.5W_hh + emb_pre
                    erow = (s % 16) * BL
                    gacts = []  # tau_i, tau_f, gt, tau_o
                    for n in range(4):
                        nsl = bass.ts(n, D)
                        ps_g = ps_g_pool.tile([BL, D], F32, tag="g",
                                              name=f"psg{s}_{n}")
                        for kc in range(KC):
                            nc.tensor.matmul(ps_g[:], ctx2T[:, kc, :],
                                             wcat_sb[:, kc, nsl],
                                             start=(kc == 0), stop=False)
                        for kc in range(KC):
                            nc.tensor.matmul(ps_g[:], hT[:, kc, :],
                                             wcat_sb[:, KC + kc, nsl],
                                             start=False, stop=False)
                        nc.tensor.matmul(ps_g[:],
                                         idsel[:, erow:erow + BL],
                                         emb_pre[:, s // 16, nsl],
                                         start=False, stop=True)
                        ga = t2k.tile([BL, D], F32, tag="t2k", name=f"ga{s}_{n}")
                        nc.scalar.activation(ga[:], ps_g[:], Tanh,
                                             scale=(1.0 if n == 2 else 0.5))
                        gacts.append(ga)
                    tau_i, tau_f, gt, tau_o = gacts

                    # 22..27) LSTM cell in tanh form
                    t_a = t2k.tile([BL, D], F32, tag="t2k", name=f"ta{s}")
                    nc.vector.scalar_tensor_tensor(t_a[:], tau_f[:], 1.0,
                                                   c_prev[:], op0=Add, op1=Mult)
                    t_b = t2k.tile([BL, D], F32, tag="t2k", name=f"tb{s}")
                    nc.vector.scalar_tensor_tensor(t_b[:], tau_i[:], 1.0,
                                                   gt[:], op0=Add, op1=Mult)
                    s2 = t2k.tile([BL, D], F32, tag="t2k", name=f"s2{s}")
                    nc.vector.tensor_add(s2[:], t_a[:], t_b[:])
                    nc.vector.tensor_scalar_mul(c_next[:], s2[:], 0.5)
                    tc2 = t2k.tile([BL, D], F32, tag="t2k", name=f"tc2{s}")
                    nc.scalar.activation(tc2[:], s2[:], Tanh, scale=0.5)
                    h2 = t2k.tile([BL, D], F32, tag="t2k", name=f"h2{s}")
                    nc.vector.scalar_tensor_tensor(h2[:], tau_o[:], 1.0, tc2[:],
                                                   op0=Add, op1=Mult)

                    # 28) h2 -> H_all[:, :, s+1, :]
                    ps_tr_h = ps_small.tile([128, KC, BL], F32, tag="small",
                                            name=f"pstrh{s}")
                    for c in range(KC):
                        nc.tensor.transpose(ps_tr_h[:, c, :], h2[:, bass.ts(c, 128)],
                                            ident[:])
                    nc.vector.tensor_copy(H_all[:, :, s + 1, :], ps_tr_h[:])

                    # interleave fc group-0 chunks once its H rows exist
                    if s >= 16:
                        quota = 5 if s < S - 1 else NVC - fc_g0_next
                        for _ in range(quota):
                            if fc_g0_next < NVC:
                                emit_fc_chunk(fc_g0_next, FC_G0[0], FC_G0[1], 0)
                                fc_g0_next += 1

                # ---------------- fc tail: rows 17..29 ----------------
                for ci in range(NVC):
                    emit_fc_chunk(ci, FC_G1[0], FC_G1[1], 128)

    nc.compile()
    return nc


def _prep_core_inputs(inputs, k):
    """Host-side marshalling for core k (samples 8k..8k+8)."""
    f32 = np.float32
    bs = slice(BL * k, BL * (k + 1))
    feats = np.ascontiguousarray(inputs["encoder_feats"][bs]).astype(f32)
    pooled = np.ascontiguousarray(inputs["encoder_pooled"][bs]).astype(f32)
    caps = np.asarray(inputs["captions"][bs])

    d = {}
    ft = feats.transpose(2, 0, 1).reshape(KC, 128, BP)
    d["featsT"] = np.ascontiguousarray(ft.transpose(1, 0, 2))
    fp = np.zeros((128, 2 * BL, ENC), f32)
    for b in range(BL):
        fp[0:128, 2 * b] = feats[b, 0:128]
        fp[0:P - 128, 2 * b + 1] = feats[b, 128:P]
    d["feats_p256"] = fp
    d["pooledT"] = np.ascontiguousarray(
        pooled.T.reshape(KC, 128, BL).transpose(1, 0, 2))
    emb = np.asarray(inputs["emb_table"], f32)[caps[:, :S]]      # (8, 29, 512)
    et = np.zeros((128, KC, 32, BL), f32)
    g = emb.transpose(2, 1, 0).reshape(KC, 128, S, BL)
    et[:, :, 0:S, :] = g.transpose(1, 0, 2, 3)
    d["embT"] = et
    return d


def _prep_shared_inputs(inputs):
    f32 = np.float32
    bf16 = ml_dtypes.bfloat16

    def rows(w, dt=f32):  # (512, N) -> [128, 4, N]
        return np.ascontiguousarray(
            np.asarray(w, f32).reshape(KC, 128, -1).transpose(1, 0, 2)).astype(dt)

    d = {}
    d["Wd"] = rows(0.5 * np.asarray(inputs["Wd_att"], f32), bf16)
    d["wf"] = rows(inputs["wf_att"])
    wih2 = 0.5 * np.asarray(inputs["W_ih"][E:], f32)
    whh = 0.5 * np.asarray(inputs["W_hh"], f32)
    wcat = np.concatenate([wih2, whh], 0)                        # (1024, 2048)
    d["Wcat"] = np.ascontiguousarray(
        wcat.reshape(2 * KC, 128, 4 * D).transpose(1, 0, 2)).astype(bf16)
    d["WihE"] = rows(inputs["W_ih"][:E])
    d["Wbeta"] = rows(0.5 * np.asarray(inputs["W_beta"], f32), bf16)
    d["Winih"] = rows(inputs["W_init_h"])
    d["Winic"] = rows(inputs["W_init_c"])
    d["We"] = rows(inputs["We_att"])
    wfc = rows(0.5 * np.asarray(inputs["W_fc"], f32))            # (128, 4, 30000)
    d["Wfc"] = np.ascontiguousarray(
        wfc.reshape(128, KC, NVC, VCHUNK).transpose(2, 0, 1, 3)).astype(bf16)
    d["bihh"] = (np.asarray(inputs["b_ih"], f32)
                 + np.asarray(inputs["b_hh"], f32)).reshape(1, -1)
    d["binih"] = np.asarray(inputs["b_init_h"], f32).reshape(1, -1)
    d["binic"] = np.asarray(inputs["b_init_c"], f32).reshape(1, -1)
    d["bd_til"] = np.ascontiguousarray(
        np.asarray(inputs["bd_att"], f32).reshape(KC, 128).T)
    d["be_til"] = np.ascontiguousarray(
        np.asarray(inputs["be_att"], f32).reshape(KC, 128).T)
    d["ones128"] = np.ones((1, 128), f32)
    return d


_NC_CACHE = {}


def _get_program(with_biases=False):
    if with_biases not in _NC_CACHE:
        _NC_CACHE[with_biases] = build_program(with_biases)
    return _NC_CACHE[with_biases]


def build_for_run(inputs):
    with_biases = bool(np.any(np.asarray(inputs["b_beta"], np.float32)))
    nc = _get_program(with_biases)
    shared = _prep_shared_inputs(inputs)
    if with_biases:
        shared["bbeta"] = np.asarray(inputs["b_beta"], np.float32).reshape(1, -1)
    in_maps = []
    for k in range(NCORES):
        m = dict(shared)
        m.update(_prep_core_inputs(inputs, k))
        in_maps.append(m)
    return nc, in_maps


def run_on_device(inputs, trace=False, **kw):
    nc, in_maps = build_for_run(inputs)
    return run_bass_kernel_spmd(nc, in_maps, list(range(NCORES)), trace=trace, **kw)


def _assemble_core(buf):
    """buf [NVC, FC_ROWS, VCHUNK] -> [BL, T, V] (t=0 row zero)."""
    out = np.zeros((BL, T, V), np.float32)
    g0 = buf[:, 0:128, :].reshape(NVC, 16, BL, VCHUNK)
    out[:, 1:17, :] = g0.transpose(2, 1, 0, 3).reshape(BL, 16, V)
    g1 = buf[:, 128:128 + 13 * BL, :].reshape(NVC, 13, BL, VCHUNK)
    out[:, 17:30, :] = g1.transpose(2, 1, 0, 3).reshape(BL, 13, V)
    return out


def kernel(**inputs) -> np.ndarray:
    res = run_on_device(inputs)
    parts = [_assemble_core(res.results[k]["out_chunks"]) for k in range(NCORES)]
    out = np.concatenate(parts, axis=0)
    b_fc = np.asarray(inputs["b_fc"], np.float32).reshape(1, 1, V)
    out[:, 1:, :] += b_fc
    return out
